# revision 68
# baseline (speedup 1.0000x reference)
"""Neural CDE forward pass on 8 Trainium2 NeuronCores.

Model (reference): z0 = coeffs[:,0]@W_init+b_init; RK4 scan over T-1=99 grid
intervals of dz = f(z) dX with f = MLP(64->128->128->128->512) -> tanh ->
reshape [H,C], contracted with dX/dt; then logits/loss/accuracy readout.

Sharding: pure data parallel over batch (2048 -> 8x256). Each core runs the
full scan on its shard; tiny readout done on host from the final z.

Device layout (per core, batch shard BS=256, split into 2 subtiles of 128):
  - Features on partitions: z [64, *], hidden h [128, *]; fp16 weights and
    activations (1 cyc/row matmuls, DVE 2x mode), fp32 PSUM accumulation,
    fp32 carried z.
  - The [128->512] output layer per subtile is one [128, 512] PSUM tile in
    4 column chunks of 128, columns permuted so chunk j / partition p'
    holds output feature (h=p'%64, c=2j+(p'>=64)); bias-seeded by a K=4
    selector matmul. The C=8 contraction is an fp16 multiply with a
    replicated dX/dt tile (host fp16, DMA-streamed per step).
  - The RK4 stage input zs is NEVER materialized: the next stage's first
    MLP layer is accumulated directly in PSUM as W_in^T zs = W_in^T z +
    a*dt * sum_j WF^T prod_j with WF = s_fold @ W_in (4 matmuls); the
    same trick with wgt-scaled folds of all 4 stage prods carries
    W_in^T z across step boundaries, and dt-scaled s_fold folds accumulate
    the raw RK4 delta (zps) for the fp32 z master + fp16 z copy, updated
    once per step.
  - The two batch subtiles run the entire scan as independent dependency
    chains, software-pipelined with a half-stage offset (emission order
    pins the phase): one chain's output layer (Act/DVE) overlaps the
    other's MLP (PE/DVE). Relus: chain 0 on DVE, chain 1 on the Act
    engine (tensor ops on Pool/gpsimd cannot touch PSUM on real HW).
"""

import numpy as np
from contextlib import ExitStack

from concourse import bacc, mybir
import concourse.tile as tile
from concourse.bass_utils import run_bass_kernel_spmd

N_CORES = 8
B, T, C, H, HH, O = 2048, 100, 8, 64, 128, 10
BS = B // N_CORES   # 256 batch rows per core
SB = BS // 2        # 128 per subtile
F32 = mybir.dt.float32
F16 = mybir.dt.float16

ADD = mybir.AluOpType.add
MAX = mybir.AluOpType.max
MULT = mybir.AluOpType.mult
TANH = mybir.ActivationFunctionType.Tanh
RELU = mybir.ActivationFunctionType.Relu

# engine assignment for the 6 relu slots (layer, subtile) and state updates
# "v" = DVE vector, "a" = Activation. NOTE: Pool/gpsimd cannot access PSUM
# on real TRN2 hardware, so PSUM-reading ops must be on DVE or Act.
RELU_ENG = {(0, 0): "v", (0, 1): "a",
            (1, 0): "v", (1, 1): "a",
            (2, 0): "v", (2, 1): "a"}
ZACC_ENG = "v"
ZH_ENG = "v"


def _build(nsteps, dts):
    """Build + compile the per-core Bass program. dts: python floats [nsteps]."""
    nc = bacc.Bacc("TRN2", target_bir_lowering=False, debug=False,
                   num_devices=N_CORES)

    def din(name, shape, dt=F16):
        return nc.dram_tensor(name, shape, dt, kind="ExternalInput").ap()

    z0f_d = din("z0f", [H, BS], F32)
    z0h_d = din("z0h", [H, BS])
    drep_d = din("drep", [nsteps, 128, 4 * BS])
    w_in_d = din("w_in", [H, HH])
    w_h0_d = din("w_h0", [HH, HH])
    w_h1_d = din("w_h1", [HH, HH])
    w_out_d = din("w_out", [HH, 4 * HH])
    sw6_d = din("sw6", [HH, H])         # dt/6 * s_fold
    sw3_d = din("sw3", [HH, H])         # dt/3 * s_fold
    wf_h_d = din("wf_h", [HH, HH])      # 0.5*dt * (s_fold @ W_in)
    wf_f_d = din("wf_f", [HH, HH])      # dt * (s_fold @ W_in)
    wf_6_d = din("wf_6", [HH, HH])      # dt/6 * (s_fold @ W_in)
    wf_3_d = din("wf_3", [HH, HH])      # dt/3 * (s_fold @ W_in)
    b4_d = din("b4", [4, HH])
    sel4_d = din("sel4", [4, 512])
    b_in_d = din("b_in", [HH, 1], F32)
    b_h0_d = din("b_h0", [HH, 1], F32)
    b_h1_d = din("b_h1", [HH, 1], F32)
    zT_d = nc.dram_tensor("zT", [H, BS], F32, kind="ExternalOutput").ap()

    with tile.TileContext(nc) as tc, ExitStack() as ctx:
        const = ctx.enter_context(tc.tile_pool(name="const", bufs=1))

        def load(ap_dram, shape, dt=F16):
            t = const.tile(shape, dt, tag=ap_dram.name)
            nc.sync.dma_start(t[:], ap_dram)
            return t

        w_in = load(w_in_d, [H, HH])
        w_h0 = load(w_h0_d, [HH, HH])
        w_h1 = load(w_h1_d, [HH, HH])
        w_out = load(w_out_d, [HH, 4 * HH])
        sw6 = load(sw6_d, [HH, H])
        sw3 = load(sw3_d, [HH, H])
        wf_h = load(wf_h_d, [HH, HH])
        wf_f = load(wf_f_d, [HH, HH])
        wf_6 = load(wf_6_d, [HH, HH])
        wf_3 = load(wf_3_d, [HH, HH])
        b4 = load(b4_d, [4, HH])
        sel4 = load(sel4_d, [4, 512])
        b_in = load(b_in_d, [HH, 1], F32)
        b_h0 = load(b_h0_d, [HH, 1], F32)
        b_h1 = load(b_h1_d, [HH, 1], F32)
        b_mlp = (b_in, b_h0, b_h1)

        sb_h = ctx.enter_context(tc.tile_pool(name="h", bufs=4))
        sb_f = ctx.enter_context(tc.tile_pool(name="f", bufs=4))
        sb_p = ctx.enter_context(tc.tile_pool(name="prod", bufs=4))
        sb_z = ctx.enter_context(tc.tile_pool(name="z", bufs=4))
        sb_d = ctx.enter_context(tc.tile_pool(name="d", bufs=4))
        ps_h = ctx.enter_context(tc.tile_pool(name="psh", bufs=1, space="PSUM"))
        ps_f = ctx.enter_context(tc.tile_pool(name="psf", bufs=1, space="PSUM"))
        ps_k = ctx.enter_context(tc.tile_pool(name="psk", bufs=1, space="PSUM"))
        ps_n = ctx.enter_context(tc.tile_pool(name="psn", bufs=1, space="PSUM"))

        def eng(which):
            return {"v": nc.vector, "a": None, "p": nc.gpsimd}[which]

        def relu(which, out, in_, b_ap):
            if which == "a":
                nc.scalar.activation(out, in_, RELU, bias=b_ap)
            else:
                eng(which).tensor_scalar(out, in_, b_ap, 0.0, op0=ADD, op1=MAX)

        # carried state, per subtile: fp32 master + fp16 matmul copy
        z0_f = const.tile([H, BS], F32, tag="zf_init")
        nc.sync.dma_start(z0_f[:], z0f_d)
        z0_h = const.tile([H, BS], F16, tag="zh_init")
        nc.sync.dma_start(z0_h[:], z0h_d)
        zf_v = [z0_f[:, 0:SB], z0_f[:, SB:BS]]
        zh_v = [z0_h[:, 0:SB], z0_h[:, SB:BS]]

        # --- software-pipelined emission: the two batch subtiles run the
        # whole scan as independent chains, offset by half a stage so one
        # subtile's output layer (Act/DVE heavy) overlaps the other's MLP
        # (PE/Pool heavy). Engine queues are in-order, so emission order
        # pins the phase. ---
        d_tiles = {}

        def get_d(ti):
            if ti not in d_tiles and ti < nsteps:
                t = sb_d.tile([128, 4 * BS], F16, name=f"d{ti}", tag="d")
                nc.sync.dma_start(t[:], drep_d[ti])
                d_tiles[ti] = t
            return d_tiles.get(ti)

        class Chain:
            """One batch subtile's scan state.

            The stage input zs is never materialized: ph1(st+1) =
            W_in^T zs = [W_in^T z seed] + a*dt * sum_j WF^T prod_j with
            WF = s_fold @ W_in, accumulated directly in PSUM (phin).
            """

            def __init__(self, s, zf, zh):
                self.s = s
                self.zh = zh        # fp16 z at step start [H, SB]
                self.zacc = zf      # fp32 running RK4 accumulator
                self.h = None
                self.pf = None
                self.phin = None    # PSUM [HH, SB]: W_in^T zs of this stage
                self.prod = None
                self.prods = []
                self.ti = 0
                self.st = 0

            def seed_phin(self, finish=False):
                """Seed the NEXT stage's phin with W_in^T z (fp16). Across
                the step boundary (st==3) the wgt-scaled folds of all four
                prod tiles complete it: W_in^T z_{t+1} = W_in^T z_t +
                sum_st wgt_st WF^T prod_st."""
                self.phin = ps_n.tile([HH, SB], F32, name=f"phin{self.s}",
                                      tag=f"phin{self.s}")
                nc.tensor.matmul(self.phin[:], w_in[:], self.zh,
                                 start=True, stop=finish,
                                 skip_group_check=True)

            def p1(self):
                """MLP (from phin) + pf bias seed for stage (ti, st)."""
                s = self.s
                if s == 0 and self.st == 0:
                    get_d(self.ti)
                    get_d(self.ti + 1)
                if self.ti == 0 and self.st == 0:
                    # very first stage: phin is the seed alone
                    self.seed_phin(finish=True)
                cur_phin = self.phin
                ph2 = ps_h.tile([HH, 2 * SB], F32, name=f"ph{s}", tag=f"ph{s}")
                h1 = sb_h.tile([HH, SB], F16, name=f"h1{s}", tag=f"h1{s}")
                relu(RELU_ENG[(0, s)], h1[:], cur_phin[:], b_in[:])
                nc.tensor.matmul(ph2[:, 0:SB], w_h0[:], h1[:],
                                 start=True, stop=True, skip_group_check=True)
                h2 = sb_h.tile([HH, SB], F16, name=f"h2{s}", tag=f"h2{s}")
                relu(RELU_ENG[(1, s)], h2[:], ph2[:, 0:SB], b_h0[:])
                nc.tensor.matmul(ph2[:, SB:2 * SB], w_h1[:], h2[:],
                                 start=True, stop=True, skip_group_check=True)
                h3 = sb_h.tile([HH, SB], F16, name=f"h3{s}", tag=f"h3{s}")
                relu(RELU_ENG[(2, s)], h3[:], ph2[:, SB:2 * SB], b_h1[:])
                self.h = h3[:]
                # next stage's phin seed, emitted late so the MLP matmuls
                # win the PE queue; folds added in p2b
                self.seed_phin()
                self.pf = ps_f.tile([128, 4 * SB], F32, name=f"pf{s}",
                                    tag=f"pf{s}")
                nc.tensor.matmul(self.pf[:], b4[:], sel4[:],
                                 start=True, stop=False, skip_group_check=True)

            def p2a(self):
                """Output-layer matmuls + tanh + d-multiply."""
                s = self.s
                d_t = d_tiles[self.ti]
                f_sb = sb_f.tile([128, 4 * SB], F16, name=f"f{s}", tag=f"f{s}")
                self.prod = sb_p.tile([128, 4 * SB], F16, name=f"prod{s}",
                                      tag=f"prod{s}")
                with tc.high_priority():
                    for j in range(4):
                        nc.tensor.matmul(
                            self.pf[:, SB * j:SB * (j + 1)],
                            w_out[:, 128 * j:128 * (j + 1)], self.h,
                            start=False, stop=(j == 3),
                            skip_group_check=True)
                nc.scalar.activation(f_sb[:], self.pf[:], TANH)
                wf = (wf_h, wf_h, wf_f, None)[self.st]
                for half in range(2):
                    lo, hi = 256 * half, 256 * (half + 1)
                    nc.vector.tensor_tensor(
                        self.prod[:, lo:hi], f_sb[:, lo:hi],
                        d_t[:, 512 * s + lo:512 * s + hi], op=MULT)
                    if wf is not None:
                        with tc.high_priority():
                            for j in (2 * half, 2 * half + 1):
                                nc.tensor.matmul(
                                    self.phin[:], wf[:],
                                    self.prod[:, SB * j:SB * (j + 1)],
                                    start=False, stop=(j == 3),
                                    skip_group_check=True)

            def p2b(self):
                """WF folds into next phin; wgt-scaled folds accumulate the
                RK4 delta in PSUM (zps) across the whole step; advance."""
                s, st = self.s, self.st
                if st == 0:
                    self.zps = ps_k.tile([H, SB], F32, name=f"zps{s}",
                                         tag=f"zps{s}")
                    self.prods = []
                self.prods.append(self.prod)
                if st == 3:
                    # boundary: phin(st0,t+1) += sum_st wgt_st WF^T prod_st;
                    # prod0-2 folds are ready early, only prod3's are on the
                    # critical path
                    for stt in (0, 1, 2):
                        wfc = (wf_6, wf_3, wf_3)[stt]
                        for j in range(4):
                            nc.tensor.matmul(
                                self.phin[:], wfc[:],
                                self.prods[stt][:, SB * j:SB * (j + 1)],
                                start=False, stop=False,
                                skip_group_check=True)
                    with tc.high_priority():
                        for j in range(4):
                            nc.tensor.matmul(
                                self.phin[:], wf_6[:],
                                self.prods[3][:, SB * j:SB * (j + 1)],
                                start=False, stop=(j == 3),
                                skip_group_check=True)
                sw = sw6 if st in (0, 3) else sw3
                for j in range(4):
                    nc.tensor.matmul(self.zps[:], sw[:],
                                     self.prod[:, SB * j:SB * (j + 1)],
                                     start=(st == 0 and j == 0),
                                     stop=(st == 3 and j == 3),
                                     skip_group_check=True)

                if st == 3:
                    # step boundary: fp16 state for the next step's seeds,
                    # fp32 master carry
                    nzh = sb_z.tile([H, SB], F16, name=f"nzh{s}",
                                    tag=f"zh{s}")
                    eng(ZH_ENG).scalar_tensor_tensor(
                        nzh[:], self.zps[:], 1.0, self.zacc,
                        op0=MULT, op1=ADD)
                    self.zh = nzh[:]
                    zan = sb_z.tile([H, SB], F32, name=f"za{s}", tag=f"za{s}")
                    eng(ZACC_ENG).scalar_tensor_tensor(
                        zan[:], self.zps[:], 1.0, self.zacc,
                        op0=MULT, op1=ADD)
                    self.zacc = zan[:]
                    self.ti += 1
                    self.st = 0
                else:
                    self.st += 1

        A = Chain(0, zf_v[0], zh_v[0])
        Bc = Chain(1, zf_v[1], zh_v[1])
        nstage = nsteps * 4
        # prologue: A runs half a stage ahead
        A.p1()
        A.p1_count = 1
        Bc.p1_count = 0
        for k in range(nstage * 2):
            lead, trail = (A, Bc) if k % 2 == 0 else (Bc, A)
            lead.p2a()
            if trail.p1_count < nstage:
                trail.p1()
                trail.p1_count += 1
            lead.p2b()
        nc.sync.dma_start(zT_d[:, 0:SB], A.zacc)
        nc.sync.dma_start(zT_d[:, SB:BS], Bc.zacc)

    nc.compile()
    return nc


def _prep_inputs(coeffs, times, W_init, b_init, W_in, b_in, W_h, b_h,
                 W_out, b_out, nsteps):
    """Host-side constants + per-core shards."""
    coeffs = np.asarray(coeffs, np.float32)
    times = np.asarray(times, np.float32)
    dts_full = np.diff(times)
    dxdt = (coeffs[:, 1:, :] - coeffs[:, :-1, :]) / dts_full[None, :, None]
    dts = dts_full[:nsteps]
    dxdt = dxdt[:, :nsteps, :]

    z0 = coeffs[:, 0, :] @ np.asarray(W_init, np.float32) + np.asarray(b_init, np.float32)
    z0 = np.ascontiguousarray(z0.T)  # [H, B]

    p = np.arange(128)
    j = np.arange(4)
    c_idx = 2 * j[None, :] + (p[:, None] >= 64)          # [128, 4]
    col = (p[:, None] % 64) * 8 + c_idx                  # [128, 4] output col

    W_out = np.asarray(W_out, np.float32)                # [HH, 512]
    b_out = np.asarray(b_out, np.float32)                # [512]
    w_out_perm = np.ascontiguousarray(
        W_out[:, col.T.reshape(-1)]).astype(np.float16)  # [HH, (j,p') 512]
    b4 = b_out[col.T].astype(np.float16)                 # [4, 128]

    sel4 = np.zeros((4, 512), np.float16)
    for k in range(4):
        sel4[k, 128 * k:128 * (k + 1)] = 1.0

    s_fold = (p[:, None] % 64 == np.arange(H)[None, :]).astype(np.float32)

    W_in_f = np.asarray(W_in, np.float32)
    dt_nom = float(np.mean(np.diff(np.asarray(times, np.float32))))
    wf = np.tile(W_in_f, (2, 1))                         # [128, 128]
    wf_h = (0.5 * dt_nom * wf).astype(np.float16)
    wf_f = (dt_nom * wf).astype(np.float16)
    sw6 = (dt_nom / 6.0 * s_fold).astype(np.float16)
    sw3 = (dt_nom / 3.0 * s_fold).astype(np.float16)
    wf_6 = (dt_nom / 6.0 * wf).astype(np.float16)
    wf_3 = (dt_nom / 3.0 * wf).astype(np.float16)

    W_h = np.asarray(W_h, np.float32)
    b_h = np.asarray(b_h, np.float32)
    consts = {
        "w_in": np.asarray(W_in, np.float32).astype(np.float16),
        "w_h0": W_h[0].astype(np.float16),
        "w_h1": W_h[1].astype(np.float16),
        "w_out": w_out_perm,
        "sw6": sw6, "sw3": sw3,
        "wf_h": wf_h, "wf_f": wf_f, "wf_6": wf_6, "wf_3": wf_3,
        "b4": b4, "sel4": sel4,
        "b_in": np.asarray(b_in, np.float32).reshape(HH, 1).copy(),
        "b_h0": b_h[0].reshape(HH, 1).copy(),
        "b_h1": b_h[1].reshape(HH, 1).copy(),
    }

    in_maps = []
    for ci in range(N_CORES):
        bs, be = ci * BS, (ci + 1) * BS
        dx_t = dxdt[bs:be].transpose(1, 2, 0)            # [nsteps, C, BS]
        drep = dx_t[:, c_idx, :]                         # [nsteps, 128, 4, BS]
        # reorder columns (j, subtile s, b) -> (s, j, b)
        drep = drep.reshape(nsteps, 128, 4, 2, SB).transpose(0, 1, 3, 2, 4)
        drep = np.ascontiguousarray(
            drep.reshape(nsteps, 128, 4 * BS)).astype(np.float16)
        m = dict(consts)
        m["z0f"] = np.ascontiguousarray(z0[:, bs:be])
        m["z0h"] = np.ascontiguousarray(z0[:, bs:be]).astype(np.float16)
        m["drep"] = drep
        in_maps.append(m)
    return in_maps, dts


_CACHE = {}


def _get_nc(nsteps, dts_key, dts):
    key = (nsteps, dts_key)
    if key not in _CACHE:
        _CACHE[key] = _build(nsteps, dts)
    return _CACHE[key]


def run_scan(coeffs, times, W_init, b_init, W_in, b_in, W_h, b_h, W_out, b_out,
             nsteps=None):
    """Run the device scan; returns zT [B, H] float32."""
    times = np.asarray(times, np.float32)
    if nsteps is None:
        nsteps = len(times) - 1
    in_maps, dts = _prep_inputs(coeffs, times, W_init, b_init, W_in, b_in,
                                W_h, b_h, W_out, b_out, nsteps)
    nc = _get_nc(nsteps, dts.tobytes(), dts)
    res = run_bass_kernel_spmd(nc, in_maps, core_ids=list(range(N_CORES)))
    zT = np.concatenate([res.results[ci]["zT"] for ci in range(N_CORES)],
                        axis=1)                          # [H, B]
    return np.ascontiguousarray(zT.T)


def kernel(coeffs, y, times, W_init, b_init, W_in, b_in, W_h, b_h,
           W_out, b_out, W_read, b_read):
    zT = run_scan(coeffs, times, W_init, b_init, W_in, b_in, W_h, b_h,
                  W_out, b_out)
    y = np.asarray(y)
    logits = (zT.astype(np.float64) @ np.asarray(W_read, np.float64)
              + np.asarray(b_read, np.float64))          # [B, O]
    m = logits.max(axis=1, keepdims=True)
    logp = logits - (m + np.log(np.exp(logits - m).sum(axis=1, keepdims=True)))
    loss = np.float32(-logp[np.arange(B), y].mean())
    acc = np.float32((logits.argmax(axis=1) == y).sum())
    return loss, acc
